# revision 1
# baseline (speedup 1.0000x reference)
"""Trainium2 Bass kernel for an AttentionBlock (GroupNorm + single-head
full N^2 attention + output projection + residual), data-parallel over
batch: 8 samples on 8 NeuronCores, no collectives.

Shapes (hardcoded): x [8, 256, 64, 64]; weights [256, 256]; biases [256].
Per core: one batch sample, x viewed as [C=256, N=4096] channel-major.

Per-core pipeline (fp8 DoubleRow matmuls: 2 k-tiles of 128 contracted per
instruction -> full C=256 contraction per matmul at double fp8 rate):
  1. GroupNorm (8 groups) in C-major layout: per-partition bn_stats,
     cross-partition group reduction via tiny constant matmuls, applied as
     per-partition scale/bias. Rounded tokens t8 (fp8e4) feed all matmuls;
     the fp32 residual out = x*s + proj' is applied in the epilogue via
     scalar_tensor_tensor, and the bias b' = b_gn + bp + Wp bv rides the
     V3 psum as a ones x b'-row outer product.
  2. Wq/Wk fold: scores = t A t^T with A = Wq^T Wk (computed on-chip,
     stored dual fp8 hi+lo). The bq bias becomes a +Wk^T bq column on the
     q2 projection (all other bias terms are per-query constants that
     cancel in softmax). The output projection folds into V:
     v3 = t (Wp Wv)^T (dual fp8), with appended ones columns so PV also
     produces softmax denominators.
  3. Attention over 256-query chunks, transposed: S^T = t8^T q8 with keys
     on partitions. Scores for a key-block PAIR live in one 1-bank psum
     tile (4 tiles in flight -> depth-4 exp pipeline). exp is split 9:7
     per chunk between ACT (native Exp -> fp8, HW-fast) and DVE via a
     single-op Schraudolph: fp8e4 bits of exp(s*SCALE - SHIFT) equal
     sat_u8_rne(s*EA8 + EB8), and the DVE f32->u8 convert saturates on
     HW (verified), so one tensor_scalar yields PV-ready fp8 weights.
     The global SHIFT keeps exp within fp8e4 range (max 240) and cancels
     in the normalization.
  4. PV accumulates [proj | colsum] in PSUM over 16 key-block pairs
     (DoubleRow). Epilogue per 128-query block: normalize by 1/colsum
     (ACT copy with per-partition scale), TensorE-transpose back to
     C-major, one scalar_tensor_tensor out = x*s + proj', DMA out.
"""

import numpy as np

import concourse.bacc as bacc
import concourse.mybir as mybir
import concourse.tile as tile
from concourse import bass_utils

F32 = mybir.dt.float32
F32R = mybir.dt.float32r
BF16 = mybir.dt.bfloat16
FP8 = mybir.dt.float8e4
I32 = mybir.dt.int32
AF = mybir.ActivationFunctionType
OP = mybir.AluOpType
DR = mybir.MatmulPerfMode.DoubleRow

B = 8
C = 256
H = 64
W = 64
N = H * W  # 4096 tokens
G = 8  # groups
GS = C // G  # 32 channels per group
P = 128
CB = C // P  # 2 channel blocks
EPS = 1e-5
NCHUNK = 512  # query chunk (matmul moving free dim)
NJ = N // NCHUNK  # 8
MB = N // P  # 32 key blocks
NPAIR = MB // 2  # 16 key-block pairs
JJ = NCHUNK // P  # 4 query sub-blocks per chunk
SCALE = C ** (-0.5)
SHIFT = 3.5  # exp(s*SCALE - SHIFT): keeps fp8e4 range, cancels in softmax

# Schraudolph exp-as-uint8-fp8-bits constants:
#   fp8e4_bits(exp(y)) ~= sat_u8_rne(y*8*log2e + 56), y = s*SCALE - SHIFT.
# DVE f32->u8 conversion saturates on HW (verified: neg -> 0, >255 -> 255,
# RNE), so one tensor_scalar per tile produces PV-ready fp8 weights.
LOG2E = 1.4426950408889634
EA8 = float(SCALE * 8.0 * LOG2E)
EB8 = float(56.0 - SHIFT * 8.0 * LOG2E)

# Engine split for the 32 single exp tiles per chunk: "A" = ACT native exp,
# "D" = DVE single-op uint8 Schraudolph (~18:14).
TILE_SCHED = (["A", "D"] * 7 + ["A", "A"]) * 2

_CACHE: dict = {}


def build_nc(att_reps=1, exp_mode="mix"):
    """exp_mode: "mix" (PAIR_SCHED), "act", "dve" (timing calibration),
    "none" (skip exp: PV reads a constant tile; output garbage)."""
    nc = bacc.Bacc(
        "TRN2",
        target_bir_lowering=False,
        debug=False,
        enable_asserts=False,
        num_devices=B,
    )

    x_d = nc.dram_tensor("x", [C, N], F32, kind="ExternalInput")
    gamma_d = nc.dram_tensor("gamma", [C], F32, kind="ExternalInput")
    beta_d = nc.dram_tensor("beta", [C], F32, kind="ExternalInput")
    w_d = {}
    b_d = {}
    for nm in ("q", "k", "v", "p"):
        w_d[nm] = nc.dram_tensor(f"W{nm}", [C, C], F32, kind="ExternalInput")
        b_d[nm] = nc.dram_tensor(f"b{nm}", [C], F32, kind="ExternalInput")
    out_d = nc.dram_tensor("out", [C, N], F32, kind="ExternalOutput")

    ident_d = nc.inline_tensor(np.eye(P, dtype=np.float32), name="ident")
    # Group-sum selector: [P, G/CB] with 1/GS entries -> group means directly.
    gsum_np = np.zeros((P, G // CB), np.float32)
    for p in range(P):
        gsum_np[p, p // GS] = 1.0 / GS
    gsum_d = nc.inline_tensor(gsum_np, name="gsum")
    # Group-broadcast selector: [G/CB, P] with 1s.
    gbc_np = np.zeros((G // CB, P), np.float32)
    for p in range(P):
        gbc_np[p // GS, p] = 1.0
    gbc_d = nc.inline_tensor(gbc_np, name="gbc")

    from contextlib import ExitStack

    with tile.TileContext(nc) as tc:
        with ExitStack() as ctx:
            _build_tile(
                ctx, tc, x_d, gamma_d, beta_d, w_d, b_d, out_d, ident_d, gsum_d,
                gbc_d, att_reps, exp_mode,
            )
    nc.compile()
    return nc


def _build_tile(ctx, tc, x_d, gamma_d, beta_d, w_d, b_d, out_d, ident_d, gsum_d, gbc_d, att_reps=1, exp_mode="mix"):
    nc = tc.nc

    persist = ctx.enter_context(tc.tile_pool(name="persist", bufs=1))
    staging = ctx.enter_context(tc.tile_pool(name="staging", bufs=4))
    sexp = ctx.enter_context(tc.tile_pool(name="sexp", bufs=8))
    si32 = ctx.enter_context(tc.tile_pool(name="si32", bufs=4))
    sout = ctx.enter_context(tc.tile_pool(name="sout", bufs=6))
    stmp = ctx.enter_context(tc.tile_pool(name="stmp", bufs=8))
    # ps_sc: [P, NCHUNK] f32 single-block score tiles (1 bank, bufs=4)
    ps_sc = ctx.enter_context(tc.tile_pool(name="ps_sc", bufs=4, space="PSUM"))
    # ps_pv: PV accumulators (1 bank each, JJ=4 alive per chunk) + transposes
    ps_pv = ctx.enter_context(tc.tile_pool(name="ps_pv", bufs=4, space="PSUM"))

    t_cm = persist.tile([P, CB, N], F32, tag="t_cm")  # raw x, C-major
    t8 = persist.tile([P, CB, N], FP8, tag="t8")  # groupnormed tokens, fp8
    NSUB = N // 512  # bn_stats free-dim limit

    # ---- x load in 1024-col slices, spread over the 3 DMA-capable queues;
    # GN constants (gsum/gbc) lead the scalar queue, weights ride gpsimd/sync
    gsum = persist.tile([P, G // CB], F32, tag="gsum")
    nc.scalar.dma_start(out=gsum, in_=gsum_d[:, :])
    gbc = persist.tile([G // CB, P], F32, tag="gbc")
    nc.scalar.dma_start(out=gbc, in_=gbc_d[:, :])

    XS = 1024
    x_q = [nc.sync, nc.sync, nc.sync, nc.scalar, nc.scalar, nc.scalar,
           nc.gpsimd, nc.gpsimd]
    for s_ in range(8):
        cb, i_ = divmod(s_, 4)
        sl = slice(i_ * XS, (i_ + 1) * XS)
        x_q[s_].dma_start(out=t_cm[:, cb, sl], in_=x_d[cb * P : (cb + 1) * P, sl])

    # staged natural-layout weights [P, CB, C] (row r = b*128+p on partition p)
    w_stage = {}
    for nm, eng in (("q", nc.gpsimd), ("k", nc.gpsimd), ("v", nc.sync), ("p", nc.sync)):
        w_sb = staging.tile([P, CB, C], F32, tag="w_stage", name=f"w_sb_{nm}")
        eng.dma_start(out=w_sb, in_=w_d[nm][:, :].rearrange("(b p) i -> p b i", p=P))
        w_stage[nm] = w_sb

    def col_tile(dram_vec, tag, eng):
        t = persist.tile([P, CB], F32, tag=tag)
        eng.dma_start(out=t, in_=dram_vec[:].rearrange("(b p) -> p b", p=P))
        return t

    gamma_col = col_tile(gamma_d, "gamma_col", nc.scalar)
    beta_col = col_tile(beta_d, "beta_col", nc.scalar)
    ident = persist.tile([P, P], F32, tag="ident")
    nc.gpsimd.dma_start(out=ident, in_=ident_d[:, :])
    bq_col = col_tile(b_d["q"], "bq_col", nc.gpsimd)
    bv_col = col_tile(b_d["v"], "bv_col", nc.gpsimd)
    bp_col = col_tile(b_d["p"], "bp_col", nc.gpsimd)

    # ---- A = Wq^T Wk  [c, c'] as dual fp8 (hi + residual lo) ----
    a8 = persist.tile([P, CB, C], FP8, tag="a8")
    a8l = persist.tile([P, CB, C], FP8, tag="a8l")
    for cb in range(CB):
        aps = ps_sc.tile([P, C], F32, tag="ps_sc", name=f"aps_{cb}")
        for mb in range(CB):
            nc.tensor.matmul(
                aps,
                lhsT=w_stage["q"][:, mb, cb * P : (cb + 1) * P],
                rhs=w_stage["k"][:, mb, :],
                start=(mb == 0),
                stop=(mb == CB - 1),
            )
        nc.scalar.copy(out=a8[:, cb, :], in_=aps)
        nc.vector.tensor_tensor(
            out=a8l[:, cb, :], in0=aps, in1=a8[:, cb, :], op=OP.subtract
        )

    # ---- WpT via TensorE transposes: [P(m), CB(mb), C(c')] f32 ----
    wpT = persist.tile([P, CB, C], F32, tag="wpT")
    for b1 in range(CB):  # c' block (rows of Wp)
        for b2 in range(CB):  # m block
            tp = ps_sc.tile([P, P], F32, tag="ps_sc")
            nc.tensor.transpose(tp, w_stage["p"][:, b1, b2 * P : (b2 + 1) * P], ident)
            nc.scalar.copy(out=wpT[:, b2, b1 * P : (b1 + 1) * P], in_=tp)

    # ---- wvp8 = (Wp Wv)^T = Wv^T Wp^T  dual fp8 (hi + residual lo) ----
    wvp8 = persist.tile([P, CB, C], FP8, tag="wvp8")
    wvp8l = persist.tile([P, CB, C], FP8, tag="wvp8l")
    for ci_b in range(CB):
        pvp = ps_sc.tile([P, C], F32, tag="ps_sc", name=f"pvp_{ci_b}")
        for cm_b in range(CB):
            nc.tensor.matmul(
                pvp,
                lhsT=w_stage["v"][:, cm_b, ci_b * P : (ci_b + 1) * P],
                rhs=wpT[:, cm_b, :],
                start=(cm_b == 0),
                stop=(cm_b == CB - 1),
            )
        nc.scalar.copy(out=wvp8[:, ci_b, :], in_=pvp)
        nc.vector.tensor_tensor(
            out=wvp8l[:, ci_b, :], in0=pvp, in1=wvp8[:, ci_b, :], op=OP.subtract
        )

    # ---- w_col = Wk^T bq (q2 bias column), bv2_col = Wp bv ----
    w_col = persist.tile([P, CB], F32, tag="w_col")
    bv2_col = persist.tile([P, CB], F32, tag="bv2_col")
    for cb in range(CB):
        wps = ps_sc.tile([P, 1], F32, tag="ps_sc", name=f"wps_{cb}")
        for mb in range(CB):
            nc.tensor.matmul(
                wps,
                lhsT=w_stage["k"][:, mb, cb * P : (cb + 1) * P],
                rhs=bq_col[:, mb : mb + 1],
                start=(mb == 0),
                stop=(mb == CB - 1),
            )
        nc.vector.tensor_copy(out=w_col[:, cb : cb + 1], in_=wps)
        vps = ps_sc.tile([P, 1], F32, tag="ps_sc", name=f"vps_{cb}")
        for mb in range(CB):
            nc.tensor.matmul(
                vps,
                lhsT=wpT[:, mb, cb * P : (cb + 1) * P],
                rhs=bv_col[:, mb : mb + 1],
                start=(mb == 0),
                stop=(mb == CB - 1),
            )
        nc.vector.tensor_copy(out=bv2_col[:, cb : cb + 1], in_=vps)

    # ---- GroupNorm stats -> per-channel scale s_col, bias b_col ----
    gn_cols = []
    for cb in range(CB):
        xt = t_cm[:, cb, :]
        stats = stmp.tile([P, NSUB, 6], F32, tag="gn_stats")
        for s in range(NSUB):
            nc.vector.bn_stats(out=stats[:, s, :], in_=xt[:, s * 512 : (s + 1) * 512])
        mv = stmp.tile([P, 2], F32, tag="gn_mv")
        nc.vector.bn_aggr(out=mv, in_=stats)
        # stats2 = (mean_p, E[x^2]_p)
        stats2 = stmp.tile([P, 2], F32, tag="gn_stats2")
        nc.vector.tensor_copy(out=stats2[:, 0:1], in_=mv[:, 0:1])
        nc.vector.tensor_tensor(
            out=stats2[:, 1:2], in0=mv[:, 0:1], in1=mv[:, 0:1], op=OP.mult
        )
        nc.vector.tensor_add(out=stats2[:, 1:2], in0=stats2[:, 1:2], in1=mv[:, 1:2])
        # group reduce: [G/CB, 2] = gsum.T @ stats2  (means already /GS)
        gps = ps_sc.tile([G // CB, 2], F32, tag="ps_sc", name=f"gps_{cb}")
        nc.tensor.matmul(gps, lhsT=gsum, rhs=stats2, start=True, stop=True)
        # rstd_g = 1/sqrt(E2_g - mean_g^2 + eps)
        gsb = stmp.tile([G // CB, 2], F32, tag="gn_gsb")
        nc.vector.tensor_copy(out=gsb, in_=gps)
        gpack = stmp.tile([G // CB, 2], F32, tag="gn_gpack")
        nc.vector.tensor_copy(out=gpack[:, 0:1], in_=gsb[:, 0:1])
        gvar = stmp.tile([G // CB, 1], F32, tag="gn_gvar")
        nc.vector.tensor_tensor(
            out=gvar, in0=gsb[:, 0:1], in1=gsb[:, 0:1], op=OP.mult
        )
        nc.vector.tensor_tensor(
            out=gvar, in0=gsb[:, 1:2], in1=gvar, op=OP.subtract
        )
        eps_t = stmp.tile([G // CB, 1], F32, tag="gn_eps")
        nc.vector.memset(eps_t, EPS)
        nc.scalar.activation(out=gvar, in_=gvar, func=AF.Sqrt, bias=eps_t)
        nc.vector.reciprocal(out=gpack[:, 1:2], in_=gvar)
        # broadcast to channels: [P, 2] = gbc.T @ gpack
        bps = ps_sc.tile([P, 2], F32, tag="ps_sc", name=f"bps_{cb}")
        nc.tensor.matmul(bps, lhsT=gbc, rhs=gpack, start=True, stop=True)
        # s_col = rstd_c * gamma_c ; b_col = beta_c - mean_c * s_col
        s_col = stmp.tile([P, 1], F32, tag="gn_scol")
        nc.vector.tensor_tensor(
            out=s_col, in0=bps[:, 1:2], in1=gamma_col[:, cb : cb + 1], op=OP.mult
        )
        b_col = stmp.tile([P, 1], F32, tag="gn_bcol")
        nc.vector.tensor_tensor(out=b_col, in0=bps[:, 0:1], in1=s_col, op=OP.mult)
        nc.vector.tensor_tensor(
            out=b_col, in0=beta_col[:, cb : cb + 1], in1=b_col, op=OP.subtract
        )
        gn_cols.append((s_col, b_col))

    # ---- b' = b_gn + bp + Wp bv as a bf16 row [1, C] (for V3 psum fold) ----
    bsum_col = persist.tile([P, CB], F32, tag="bsum_col")
    for cb in range(CB):
        _, b_col = gn_cols[cb]
        nc.vector.tensor_add(
            out=bsum_col[:, cb : cb + 1], in0=b_col, in1=bp_col[:, cb : cb + 1]
        )
        nc.vector.tensor_add(
            out=bsum_col[:, cb : cb + 1],
            in0=bsum_col[:, cb : cb + 1],
            in1=bv2_col[:, cb : cb + 1],
        )
    bsum_bf = persist.tile([P, CB], BF16, tag="bsum_bf")
    nc.vector.tensor_copy(out=bsum_bf, in_=bsum_col)
    b_row = persist.tile([1, C], BF16, tag="b_row")
    for cb in range(CB):
        nc.sync.dma_start(
            out=b_row[0:1, cb * P : (cb + 1) * P], in_=bsum_bf[:, cb : cb + 1]
        )
    ones_bf = persist.tile([1, P], BF16, tag="ones_bf")
    nc.vector.memset(ones_bf, 1.0)
    nshift_col = persist.tile([P, 1], F32, tag="nshift_col")
    nc.vector.memset(nshift_col, -SHIFT)

    # rounded fp8 tokens: t8 = x*s + b per channel block; cb0 on DVE, cb1 on
    # ScalarE run in parallel
    for sch in range(NSUB):
        asl = slice(sch * 512, (sch + 1) * 512)
        s_col0, b_col0 = gn_cols[0]
        nc.vector.tensor_scalar(
            out=t8[:, 0, asl], in0=t_cm[:, 0, asl], scalar1=s_col0,
            scalar2=b_col0, op0=OP.mult, op1=OP.add,
        )
        s_col1, b_col1 = gn_cols[1]
        nc.scalar.activation(
            out=t8[:, 1, asl], in_=t_cm[:, 1, asl], func=AF.Identity,
            bias=b_col1, scale=s_col1,
        )

    # ---- V3 storage with ones columns; q8 ----
    q8 = persist.tile([P, CB, N], FP8, tag="q8")
    v_aug = persist.tile([P, MB, C + 2], FP8, tag="v_aug")
    ones_c8 = persist.tile([P, 1], FP8, tag="ones_c8")
    nc.vector.memset(ones_c8, 1.0)
    nc.scalar.copy(out=v_aug[:, :, C : C + 2], in_=ones_c8.to_broadcast((P, MB, 2)))

    def q_chunk(ch):
        sl = slice(ch * NCHUNK, (ch + 1) * NCHUNK)
        for cb in range(CB):
            pq = ps_sc.tile([P, NCHUNK], F32, tag="ps_sc", name=f"pq_{ch}_{cb}")
            nc.tensor.matmul(
                pq,
                lhsT=a8[:, :, cb * P : (cb + 1) * P],
                rhs=t8[:, :, sl],
                start=True,
                stop=False,
                perf_mode=DR,
            )
            nc.tensor.matmul(
                pq,
                lhsT=a8l[:, :, cb * P : (cb + 1) * P],
                rhs=t8[:, :, sl],
                start=False,
                stop=True,
                perf_mode=DR,
            )
            nc.scalar.activation(
                out=q8[:, cb, sl], in_=pq, func=AF.Identity,
                bias=w_col[:, cb : cb + 1], scale=1.0,
            )

    def v_block(nb, eng):
        vps = ps_sc.tile([P, NCHUNK], F32, tag="ps_sc", name=f"vb_{nb}")
        nc.tensor.matmul(
            vps[:, 0:C],
            lhsT=t8[:, :, nb * P : (nb + 1) * P],
            rhs=wvp8[:, :, :],
            start=True,
            stop=False,
            perf_mode=DR,
        )
        nc.tensor.matmul(
            vps[:, 0:C],
            lhsT=t8[:, :, nb * P : (nb + 1) * P],
            rhs=wvp8l[:, :, :],
            start=False,
            stop=False,
            perf_mode=DR,
        )
        # + ones x b' (residual+proj bias, recovered by the normalize fold)
        nc.tensor.matmul(
            vps[:, 0:C], lhsT=ones_bf, rhs=b_row, start=False, stop=True
        )
        if eng == "A":
            nc.scalar.copy(out=v_aug[:, nb, 0:C], in_=vps[:, 0:C])
        else:
            nc.vector.tensor_copy(out=v_aug[:, nb, 0:C], in_=vps[:, 0:C])

    ex_const = None
    if exp_mode == "none":
        ex_const = persist.tile([P, 2, NCHUNK], FP8, tag="ex_const")
        nc.vector.memset(ex_const, 0.25)

    def produce_pair(j, bp_, with_v):
        if with_v:
            v_block(2 * bp_, "A" if bp_ % 2 else "D")
            v_block(2 * bp_ + 1, "D" if bp_ % 2 else "A")
        jsl = slice((j % NJ) * NCHUNK, (j % NJ + 1) * NCHUNK)
        ex = None if exp_mode == "none" else sexp.tile(
            [P, 2, NCHUNK], FP8, tag="exp", name=f"ex_{j}_{bp_}"
        )
        for h in range(2):
            i_ = 2 * bp_ + h
            ss = ps_sc.tile([P, NCHUNK], F32, tag="ps_sc", name=f"ss_{j}_{i_}")
            nc.tensor.matmul(
                ss,
                lhsT=t8[:, :, i_ * P : (i_ + 1) * P],
                rhs=q8[:, :, jsl],
                start=True,
                stop=True,
                perf_mode=DR,
            )
            if exp_mode == "none":
                continue
            eng = (TILE_SCHED[i_] if exp_mode == "mix"
                   else ("A" if exp_mode == "act" else "D"))
            if eng == "A":
                nc.scalar.activation(
                    out=ex[:, h, :], in_=ss, func=AF.Exp, scale=SCALE,
                    bias=nshift_col,
                )
            else:
                nc.vector.tensor_scalar(
                    out=ex[:, h, :].bitcast(mybir.dt.uint8), in0=ss,
                    scalar1=EA8, scalar2=EB8, op0=OP.mult, op1=OP.add,
                )
        return ex_const if exp_mode == "none" else ex

    def pv_mm(pv_ps, bp_, ex_pair):
        for jj in range(JJ):
            nc.tensor.matmul(
                pv_ps[jj],
                lhsT=ex_pair[:, :, jj * P : (jj + 1) * P],
                rhs=v_aug[:, 2 * bp_ : 2 * bp_ + 2, :],
                start=(bp_ == 0),
                stop=(bp_ == NPAIR - 1),
                perf_mode=DR,
            )

    def epilogue(j, pv_ps):
        jsl = slice((j % NJ) * NCHUNK, (j % NJ + 1) * NCHUNK)
        obs = [
            sout.tile([P, NCHUNK], F32, tag="out", name=f"ob_{j}_{co}")
            for co in range(CB)
        ]
        for jj in range(JJ):
            rec = stmp.tile([P, 1], F32, tag="rec")
            nc.vector.reciprocal(out=rec, in_=pv_ps[jj][:, C : C + 1])
            anm = stmp.tile([P, C], F32, tag="anm")
            nc.scalar.activation(
                out=anm, in_=pv_ps[jj][:, 0:C], func=AF.Copy, scale=rec
            )
            for co in range(CB):
                tp = ps_pv.tile([P, P], F32, tag="ps_pv")
                nc.tensor.transpose(tp, anm[:, co * P : (co + 1) * P], ident)
                nbase = (j % NJ) * NCHUNK + jj * P
                s_col, _ = gn_cols[co]
                nc.vector.scalar_tensor_tensor(
                    out=obs[co][:, jj * P : (jj + 1) * P],
                    in0=t_cm[:, co, nbase : nbase + P],
                    scalar=s_col,
                    in1=tp,
                    op0=OP.mult,
                    op1=OP.add,
                )
        for co in range(CB):
            nc.sync.dma_start(out=out_d[co * P : (co + 1) * P, jsl], in_=obs[co])

    def pv_tiles(j):
        return [
            ps_pv.tile([P, C + 2], F32, tag="ps_pv", name=f"pv_ps_{j}_{jj}")
            for jj in range(JJ)
        ]

    # ---- chunk 0, interleaved with V3 production ----
    LA = 2  # pair lookahead (2 pairs = 4 single-bank score tiles)
    q_chunk(0)
    pv_ps0 = pv_tiles(0)
    exs = {bp_: produce_pair(0, bp_, True) for bp_ in range(LA)}
    for bp_ in range(NPAIR):
        pv_mm(pv_ps0, bp_, exs.pop(bp_))
        if bp_ + LA < NPAIR:
            exs[bp_ + LA] = produce_pair(0, bp_ + LA, True)
    q_chunk(1)
    epilogue(0, pv_ps0)

    # ---- remaining chunks ----
    for j in range(1, NJ * att_reps):
        if 0 < j % NJ and j % NJ < NJ - 1 and j < NJ:
            q_chunk(j + 1)
        pv_ps = pv_tiles(j)
        exs = {bp_: produce_pair(j, bp_, False) for bp_ in range(LA)}
        for bp_ in range(NPAIR):
            pv_mm(pv_ps, bp_, exs.pop(bp_))
            if bp_ + LA < NPAIR:
                exs[bp_ + LA] = produce_pair(j, bp_ + LA, False)
        epilogue(j, pv_ps)


def kernel(x, gamma, beta, Wq, bq, Wk, bk, Wv, bv, Wp, bp):
    if "nc" not in _CACHE:
        _CACHE["nc"] = build_nc()
    nc = _CACHE["nc"]

    x = np.ascontiguousarray(np.asarray(x, dtype=np.float32)).reshape(B, C, N)
    common = {
        "gamma": np.asarray(gamma, np.float32),
        "beta": np.asarray(beta, np.float32),
        "Wq": np.asarray(Wq, np.float32),
        "bq": np.asarray(bq, np.float32),
        "Wk": np.asarray(Wk, np.float32),
        "bk": np.asarray(bk, np.float32),
        "Wv": np.asarray(Wv, np.float32),
        "bv": np.asarray(bv, np.float32),
        "Wp": np.asarray(Wp, np.float32),
        "bp": np.asarray(bp, np.float32),
    }
    in_maps = [{"x": x[b], **common} for b in range(B)]
    res = bass_utils.run_bass_kernel_spmd(nc, in_maps, core_ids=list(range(B)))
    out = np.stack([res.results[b]["out"] for b in range(B)])
    return out.reshape(B, C, H, W)



# revision 53
# speedup vs baseline: 1.0672x; 1.0672x over previous
"""Trainium2 Bass kernel for an AttentionBlock (GroupNorm + single-head
full N^2 attention + output projection + residual), data-parallel over
batch: 8 samples on 8 NeuronCores, no collectives.

Shapes (hardcoded): x [8, 256, 64, 64]; weights [256, 256]; biases [256].
Per core: one batch sample, x viewed as [C=256, N=4096] channel-major.

Per-core pipeline (fp8 DoubleRow matmuls: 2 k-tiles of 128 contracted per
instruction -> full C=256 contraction per matmul at double fp8 rate):
  0. x loads as bf16 via casting gpsimd/SWDGE DMAs, halving the 4MB
     x traffic on the serial DMA path that gates the setup phase. The
     bf16 rounding is invisible downstream: GN stats average it out over
     131k elements/group, tokens are rounded to fp8 anyway, and the
     residual already used a bf16 copy.
  1. GroupNorm (8 groups) in C-major layout: per-partition bn_stats,
     cross-partition group reduction via tiny constant matmuls, applied as
     per-partition scale/bias. Rounded tokens t8 (fp8e4) feed all matmuls;
     computed in 1024-col ops spread over DVE/ACT/GPSIMD. The biases fold
     into matmuls: b' = b_gn + bp + Wp bv rides the V3 psum and
     w = Wk^T bq rides the q2 psum, each as a ones x row outer product.
  2. Wq/Wk fold: scores = t A t^T with A = Wq^T Wk (computed on-chip,
     stored dual fp8 hi+lo). The output projection folds into V:
     v3 = t (Wp Wv)^T (dual fp8), with appended ones columns so PV also
     produces softmax denominators.
  3. Attention over 512-query chunks, transposed: S^T = t8^T q8 with keys
     on partitions. Scores for a key-block PAIR live in ONE 2-bank psum
     region [P, 1024]; 3 regions rotate so a single 1024-col exp op per
     pair keeps both exp engines saturated (pair production runs LA=3
     ahead, and XLA=6 pairs of the next chunk are primed before each
     epilogue). exp alternates ACT (native Exp -> fp8) and DVE via a
     single-op Schraudolph: fp8e4 bits of exp(s*SCALE - SHIFT) equal
     sat_u8_rne(s*EA8 + EB8), and the DVE f32->u8 convert saturates on
     HW (verified), so one tensor_scalar yields PV-ready fp8 weights.
     The global SHIFT keeps exp within fp8e4 range (max 240) and cancels
     in the normalization.
  4. PV runs in two half-passes (query sub-blocks jj0/jj1 then jj2/jj3)
     over the chunk's persistent SBUF ex tiles, so only 2 PV psum banks
     are live and the score pipeline gets its 3rd region. Epilogue per
     chunk: normalize by 1/colsum per 128-query block, then a borrowed
     score region is seeded with the scaled residual s*x via a diag(s)
     bf16 matmul (reading the bf16 x directly) and the
     TensorE transposes accumulate the C-major projection on top; the
     psum -> sbuf move is then a plain engine-assignable copy, DMA out.
     The transpose/copy tail of chunk j is deferred into chunk j+1 so it
     overlaps pair production; only the last chunk runs it eagerly, split
     around its PV half-1 pass.
"""
import numpy as np

import concourse.bacc as bacc
import concourse.mybir as mybir
import concourse.tile as tile
from concourse import bass_utils

F32 = mybir.dt.float32
F32R = mybir.dt.float32r
BF16 = mybir.dt.bfloat16
FP8 = mybir.dt.float8e4
I32 = mybir.dt.int32
AF = mybir.ActivationFunctionType
OP = mybir.AluOpType
DR = mybir.MatmulPerfMode.DoubleRow

B = 8
C = 256
H = 64
W = 64
N = H * W  # 4096 tokens
G = 8  # groups
GS = C // G  # 32 channels per group
P = 128
CB = C // P  # 2 channel blocks
EPS = 1e-5
NCHUNK = 512  # query chunk (matmul moving free dim)
NJ = N // NCHUNK  # 8
MB = N // P  # 32 key blocks
NPAIR = MB // 2  # 16 key-block pairs
JJ = NCHUNK // P  # 4 query sub-blocks per chunk
SCALE = C ** (-0.5)
SHIFT = 3.5  # exp(s*SCALE - SHIFT): keeps fp8e4 range, cancels in softmax

# Schraudolph exp-as-uint8-fp8-bits constants:
#   fp8e4_bits(exp(y)) ~= sat_u8_rne(y*8*log2e + 56), y = s*SCALE - SHIFT.
# DVE f32->u8 conversion saturates on HW (verified: neg -> 0, >255 -> 255,
# RNE), so one tensor_scalar per pair region produces PV-ready fp8 weights.
LOG2E = 1.4426950408889634
EA8 = float(SCALE * 8.0 * LOG2E)
EB8 = float(56.0 - SHIFT * 8.0 * LOG2E)

# Engine split for the 16 merged exp pair-ops per chunk: "A" = ACT native
# exp, "D" = DVE single-op uint8 Schraudolph. Strict alternation keeps the
# 3-region score pipeline symmetric (each engine fires every other pair).
EXP_SCHED = ["A", "D"] * 8
# Normalize (anm) engine per query sub-block.
ANM_SCHED = ["A", "D", "A", "D"]
Q8_ENG = "A"
XLA = 6  # pairs primed into the next chunk before the epilogue ops
LA = 3  # in-chunk pair lookahead (= score region count)
QPOS = 6  # pair index in chunk j where q8 for chunk j+1 is produced
TAIL_BP = 0  # where in the next chunk the deferred transpose/stt tail runs
# t8 1024-col slices: [cb][k] engine, "P" = GPSIMD (otherwise idle).
T8_SCHED = [["D", "A", "P", "D"], ["A", "D", "P", "A"]]
T8_INLINE = False  # emit each cb's t8 inside the GN loop
DMA_MODE = "orig"  # "orig" | "wfirst_fine"
STATS_MODE = "bn"  # "bn" | "split": cb1 stats via ACT/DVE accum_out ops
GN_HIPRI = False  # schedule the GN stats chain at max priority
SOUT_BUFS = 6
STMP_BUFS = 8
EXP0_SCHED = None  # chunk-0 exp split override (None -> EXP_SCHED)
FRONT_PIN = 0  # 0=off; else scale for wait_until pins on the prep blocks
V_ENG = "opp"  # chunk-0 v_aug copy engine: "opp" (opposite of exp) | "A" | "D"

_CACHE: dict = {}


def build_nc(att_reps=1, exp_mode="mix"):
    """exp_mode: "mix" (EXP_SCHED), "act", "dve" (timing calibration),
    "none" (skip exp: PV reads a constant tile; output garbage)."""
    nc = bacc.Bacc(
        "TRN2",
        target_bir_lowering=False,
        debug=False,
        enable_asserts=False,
        num_devices=B,
    )

    x_d = nc.dram_tensor("x", [C, N], F32, kind="ExternalInput")
    gamma_d = nc.dram_tensor("gamma", [C], F32, kind="ExternalInput")
    beta_d = nc.dram_tensor("beta", [C], F32, kind="ExternalInput")
    w_d = {}
    b_d = {}
    for nm in ("q", "k", "v", "p"):
        w_d[nm] = nc.dram_tensor(f"W{nm}", [C, C], F32, kind="ExternalInput")
        b_d[nm] = nc.dram_tensor(f"b{nm}", [C], F32, kind="ExternalInput")
    out_d = nc.dram_tensor("out", [C, N], F32, kind="ExternalOutput")

    ident_d = nc.inline_tensor(np.eye(P, dtype=np.float32), name="ident")
    # Group-sum selector: [P, G/CB] with 1/GS entries -> group means directly.
    gsum_np = np.zeros((P, G // CB), np.float32)
    for p in range(P):
        gsum_np[p, p // GS] = 1.0 / GS
    gsum_d = nc.inline_tensor(gsum_np, name="gsum")
    # Group-broadcast selector: [G/CB, P] with 1s.
    gbc_np = np.zeros((G // CB, P), np.float32)
    for p in range(P):
        gbc_np[p // GS, p] = 1.0
    gbc_d = nc.inline_tensor(gbc_np, name="gbc")

    from contextlib import ExitStack

    with tile.TileContext(nc) as tc:
        with ExitStack() as ctx:
            _build_tile(
                ctx, tc, x_d, gamma_d, beta_d, w_d, b_d, out_d, ident_d, gsum_d,
                gbc_d, att_reps, exp_mode,
            )
    nc.compile()
    return nc


def _build_tile(ctx, tc, x_d, gamma_d, beta_d, w_d, b_d, out_d, ident_d, gsum_d, gbc_d, att_reps=1, exp_mode="mix"):
    nc = tc.nc

    persist = ctx.enter_context(tc.tile_pool(name="persist", bufs=1))
    staging = ctx.enter_context(tc.tile_pool(name="staging", bufs=4))
    # ex tiles live for a whole chunk (both PV half-passes) + lookahead
    sexp = ctx.enter_context(tc.tile_pool(name="sexp", bufs=NPAIR + XLA + 1))
    sout = ctx.enter_context(tc.tile_pool(name="sout", bufs=SOUT_BUFS))
    stmp = ctx.enter_context(tc.tile_pool(name="stmp", bufs=STMP_BUFS))
    # ps_sc: [P, 1024] f32 2-bank score/psum regions (bufs=3 -> 6 banks).
    # Also borrowed for q-projection psum and the epilogue transpose region.
    ps_sc = ctx.enter_context(tc.tile_pool(name="ps_sc", bufs=3, space="PSUM"))
    # ps_pv: PV accumulators, 2 live at a time (PV runs in two half-passes
    # over the persistent SBUF ex tiles: jj0/jj1 then jj2/jj3)
    ps_pv = ctx.enter_context(tc.tile_pool(name="ps_pv", bufs=2, space="PSUM"))

    t_cm = persist.tile([P, CB, N], BF16, tag="t_cm")  # raw x, bf16, C-major
    t_bf = t_cm  # epilogue residual reads the same bf16 tensor
    t8 = persist.tile([P, CB, N], FP8, tag="t8")  # groupnormed tokens, fp8
    NSUB = N // 512  # bn_stats free-dim limit

    # ---- x load in 1024-col slices over 4 DMA queues (2 each, cb0 first so
    # bn_stats can start early); GN constants (gsum/gbc) lead the scalar queue
    if not FRONT_PIN:
        gsum = persist.tile([P, G // CB], F32, tag="gsum")
        nc.scalar.dma_start(out=gsum, in_=gsum_d[:, :])
        gbc = persist.tile([G // CB, P], F32, tag="gbc")
        nc.scalar.dma_start(out=gbc, in_=gbc_d[:, :])

    # x slices: sync s0/s3/s6, scalar s1/s4/s7, gpsimd s2/s5. Weights ride
    # the fast sync/HWDGE queue interleaved with x so A/wvp8 prep can start
    # ~5us in (the gpsimd/SWDGE queue generates descriptors on the Q7 and is
    # much slower per DMA).
    XS = 1024
    w_stage = {
        nm: staging.tile([P, CB, C], F32, tag="w_stage", name=f"w_sb_{nm}")
        for nm in ("q", "k", "v", "p")
    }

    def x_load(eng, s_):
        # f32 -> bf16 cast in flight: casting DMAs are SWDGE/gpsimd-only
        cb, i_ = divmod(s_, 4)
        sl = slice(i_ * XS, (i_ + 1) * XS)
        nc.gpsimd.dma_start(out=t_cm[:, cb, sl], in_=x_d[cb * P : (cb + 1) * P, sl])

    def w_load(nm):
        nc.sync.dma_start(
            out=w_stage[nm], in_=w_d[nm][:, :].rearrange("(b p) i -> p b i", p=P)
        )

    if DMA_MODE == "wfirst_fine":
        # ident+Wq+Wk first (unblock A-prep), x in 512-col slices cb0-first,
        # Wv/Wp after (V path isn't needed until ~18us)
        ident = persist.tile([P, P], F32, tag="ident")
        nc.sync.dma_start(out=ident, in_=ident_d[:, :])
        w_load("q")
        w_load("k")
        xq3 = [nc.sync, nc.scalar, nc.gpsimd]
        for h_ in range(16):
            cb, i_ = divmod(h_, 8)
            sl = slice(i_ * 512, (i_ + 1) * 512)
            xq3[h_ % 3].dma_start(
                out=t_cm[:, cb, sl], in_=x_d[cb * P : (cb + 1) * P, sl]
            )
        w_load("v")
        w_load("p")
    elif DMA_MODE == "worder2":
        # ident+Wq+Wk lead the serial DMA device (~1.8us) so the cold fp32
        # A-prep matmul chain (~5us on PE) runs during the x load instead of
        # after it; Wv/Wp follow x (the V path isn't needed until ~18us).
        ident = persist.tile([P, P], F32, tag="ident")
        nc.sync.dma_start(out=ident, in_=ident_d[:, :])
        for nm, eng in (("q", nc.sync), ("k", nc.sync)):
            eng.dma_start(
                out=w_stage[nm], in_=w_d[nm][:, :].rearrange("(b p) i -> p b i", p=P)
            )
        for s_ in range(8):
            x_load([nc.sync, nc.scalar, nc.gpsimd][s_ % 3], s_)
        for nm, eng in (("v", nc.sync), ("p", nc.sync)):
            eng.dma_start(
                out=w_stage[nm], in_=w_d[nm][:, :].rearrange("(b p) i -> p b i", p=P)
            )
    elif DMA_MODE == "fine":
        xq3 = [nc.sync, nc.scalar, nc.gpsimd]
        for h_ in range(16):
            cb, i_ = divmod(h_, 8)
            sl = slice(i_ * 512, (i_ + 1) * 512)
            xq3[h_ % 3].dma_start(
                out=t_cm[:, cb, sl], in_=x_d[cb * P : (cb + 1) * P, sl]
            )
        for nm, eng in (("q", nc.gpsimd), ("k", nc.gpsimd), ("v", nc.sync), ("p", nc.sync)):
            eng.dma_start(
                out=w_stage[nm], in_=w_d[nm][:, :].rearrange("(b p) i -> p b i", p=P)
            )
    else:
        for s_ in range(8):
            x_load([nc.sync, nc.scalar, nc.gpsimd][s_ % 3], s_)
        for nm, eng in (("q", nc.sync), ("k", nc.sync), ("v", nc.sync), ("p", nc.sync)):
            eng.dma_start(
                out=w_stage[nm], in_=w_d[nm][:, :].rearrange("(b p) i -> p b i", p=P)
            )

    def col_tile(dram_vec, tag, eng):
        t = persist.tile([P, CB], F32, tag=tag)
        eng.dma_start(out=t, in_=dram_vec[:].rearrange("(b p) -> p b", p=P))
        return t

    if FRONT_PIN:
        # behind the x slices on the scalar queue so x1 isn't pushed to the
        # back of the serial DMA device (bn_stats #3 stall)
        gsum = persist.tile([P, G // CB], F32, tag="gsum")
        nc.scalar.dma_start(out=gsum, in_=gsum_d[:, :])
        gbc = persist.tile([G // CB, P], F32, tag="gbc")
        nc.scalar.dma_start(out=gbc, in_=gbc_d[:, :])
    gamma_col = col_tile(gamma_d, "gamma_col", nc.scalar)
    beta_col = col_tile(beta_d, "beta_col", nc.scalar)
    if DMA_MODE not in ("wfirst_fine", "worder2"):
        ident = persist.tile([P, P], F32, tag="ident")
        nc.gpsimd.dma_start(out=ident, in_=ident_d[:, :])
    bq_col = col_tile(b_d["q"], "bq_col", nc.gpsimd)
    bv_col = col_tile(b_d["v"], "bv_col", nc.gpsimd)
    bp_col = col_tile(b_d["p"], "bp_col", nc.gpsimd)

    # ---- A = Wq^T Wk  [c, c'] as dual fp8 (hi + residual lo) ----
    a8 = persist.tile([P, CB, C], FP8, tag="a8")
    a8l = persist.tile([P, CB, C], FP8, tag="a8l")
    for cb in range(CB):
        aps = ps_sc.tile([P, 2 * NCHUNK], F32, tag="ps_sc", name=f"aps_{cb}")
        for mb in range(CB):
            nc.tensor.matmul(
                aps[:, 0:C],
                lhsT=w_stage["q"][:, mb, cb * P : (cb + 1) * P],
                rhs=w_stage["k"][:, mb, :],
                start=(mb == 0),
                stop=(mb == CB - 1),
            )
        nc.scalar.copy(out=a8[:, cb, :], in_=aps[:, 0:C])
        nc.vector.tensor_tensor(
            out=a8l[:, cb, :], in0=aps[:, 0:C], in1=a8[:, cb, :], op=OP.subtract
        )

    # ---- GroupNorm stats -> per-channel scale s_col, bias b_col ----
    gn_cols = []
    from contextlib import nullcontext
    with (tc.high_priority() if GN_HIPRI else nullcontext()):
      for cb in range(CB):
        xt = t_cm[:, cb, :]
        if STATS_MODE == "split" and cb == 1:
            # cb1 stats off the DVE serial path: mean partials via ACT
            # Identity+accum (1/N folded into scale), E[x^2] partials via DVE
            # scalar_tensor_tensor+accum ((x/N) * x)
            sacc = stmp.tile([P, 8], F32, tag="gn_acc")
            scr_a = stmp.tile([P, 1024], F32, tag="gn_scr_a", bufs=1)
            scr_d = stmp.tile([P, 1024], F32, tag="gn_scr_d", bufs=1)
            for k4 in range(4):
                asl = slice(k4 * 1024, (k4 + 1) * 1024)
                nc.scalar.activation(
                    out=scr_a, in_=xt[:, asl], func=AF.Identity,
                    scale=1.0 / N, accum_out=sacc[:, k4 : k4 + 1],
                )
                nc.vector.scalar_tensor_tensor(
                    out=scr_d, in0=xt[:, asl], scalar=1.0 / N, in1=xt[:, asl],
                    op0=OP.mult, op1=OP.mult,
                    accum_out=sacc[:, 4 + k4 : 5 + k4],
                )
            stats2 = stmp.tile([P, 2], F32, tag="gn_stats2")
            h1 = stmp.tile([P, 2], F32, tag="gn_h1")
            nc.vector.tensor_add(out=h1[:, 0:1], in0=sacc[:, 0:1], in1=sacc[:, 1:2])
            nc.vector.tensor_add(out=h1[:, 1:2], in0=sacc[:, 2:3], in1=sacc[:, 3:4])
            nc.vector.tensor_add(out=stats2[:, 0:1], in0=h1[:, 0:1], in1=h1[:, 1:2])
            h2 = stmp.tile([P, 2], F32, tag="gn_h2")
            nc.vector.tensor_add(out=h2[:, 0:1], in0=sacc[:, 4:5], in1=sacc[:, 5:6])
            nc.vector.tensor_add(out=h2[:, 1:2], in0=sacc[:, 6:7], in1=sacc[:, 7:8])
            nc.vector.tensor_add(out=stats2[:, 1:2], in0=h2[:, 0:1], in1=h2[:, 1:2])
        else:
            stats = stmp.tile([P, NSUB, 6], F32, tag="gn_stats")
            for s in range(NSUB):
                nc.vector.bn_stats(out=stats[:, s, :], in_=xt[:, s * 512 : (s + 1) * 512])
            mv = stmp.tile([P, 2], F32, tag="gn_mv")
            nc.vector.bn_aggr(out=mv, in_=stats)
            # stats2 = (mean_p, E[x^2]_p)
            stats2 = stmp.tile([P, 2], F32, tag="gn_stats2")
            nc.vector.tensor_copy(out=stats2[:, 0:1], in_=mv[:, 0:1])
            nc.vector.tensor_tensor(
                out=stats2[:, 1:2], in0=mv[:, 0:1], in1=mv[:, 0:1], op=OP.mult
            )
            nc.vector.tensor_add(out=stats2[:, 1:2], in0=stats2[:, 1:2], in1=mv[:, 1:2])
        # group reduce: [G/CB, 2] = gsum.T @ stats2  (means already /GS)
        gps = ps_sc.tile([P, 2 * NCHUNK], F32, tag="ps_sc", name=f"gps_{cb}")
        nc.tensor.matmul(gps[0 : G // CB, 0:2], lhsT=gsum, rhs=stats2, start=True, stop=True)
        # rstd_g = 1/sqrt(E2_g - mean_g^2 + eps)
        gsb = stmp.tile([G // CB, 2], F32, tag="gn_gsb")
        nc.vector.tensor_copy(out=gsb, in_=gps[0 : G // CB, 0:2])
        gpack = stmp.tile([G // CB, 2], F32, tag="gn_gpack")
        nc.vector.tensor_copy(out=gpack[:, 0:1], in_=gsb[:, 0:1])
        gvar = stmp.tile([G // CB, 1], F32, tag="gn_gvar")
        nc.vector.tensor_tensor(
            out=gvar, in0=gsb[:, 0:1], in1=gsb[:, 0:1], op=OP.mult
        )
        nc.vector.tensor_tensor(
            out=gvar, in0=gsb[:, 1:2], in1=gvar, op=OP.subtract
        )
        # rstd = 1/sqrt(var+eps) (DVE pow is rejected by the ISA checker)
        eps_t = stmp.tile([G // CB, 1], F32, tag="gn_eps")
        nc.vector.memset(eps_t, EPS)
        nc.scalar.activation(out=gvar, in_=gvar, func=AF.Sqrt, bias=eps_t)
        nc.vector.reciprocal(out=gpack[:, 1:2], in_=gvar)
        # broadcast to channels: [P, 2] = gbc.T @ gpack
        bps = ps_sc.tile([P, 2 * NCHUNK], F32, tag="ps_sc", name=f"bps_{cb}")
        nc.tensor.matmul(bps[:, 0:2], lhsT=gbc, rhs=gpack, start=True, stop=True)
        # s_col = rstd_c * gamma_c ; b_col = beta_c - mean_c * s_col
        s_col = stmp.tile([P, 1], F32, tag="gn_scol")
        nc.vector.tensor_tensor(
            out=s_col, in0=bps[:, 1:2], in1=gamma_col[:, cb : cb + 1], op=OP.mult
        )
        b_col = stmp.tile([P, 1], F32, tag="gn_bcol")
        nc.vector.tensor_tensor(out=b_col, in0=bps[:, 0:1], in1=s_col, op=OP.mult)
        nc.vector.tensor_tensor(
            out=b_col, in0=beta_col[:, cb : cb + 1], in1=b_col, op=OP.subtract
        )
        gn_cols.append((s_col, b_col))
        if not T8_INLINE:
            continue
        # rounded fp8 tokens for this channel block immediately (cb0's t8
        # overlaps cb1's stats): 1024-col ops over DVE/ACT/GPSIMD
        for k in range(N // 1024):
            asl = slice(k * 1024, (k + 1) * 1024)
            eng = T8_SCHED[cb][k]
            if eng == "D":
                nc.vector.tensor_scalar(
                    out=t8[:, cb, asl], in0=t_cm[:, cb, asl], scalar1=s_col,
                    scalar2=b_col, op0=OP.mult, op1=OP.add,
                )
            elif eng == "P":
                nc.gpsimd.tensor_scalar(
                    out=t8[:, cb, asl], in0=t_cm[:, cb, asl], scalar1=s_col,
                    scalar2=b_col, op0=OP.mult, op1=OP.add,
                )
            else:
                nc.scalar.activation(
                    out=t8[:, cb, asl], in_=t_cm[:, cb, asl], func=AF.Identity,
                    bias=b_col, scale=s_col,
                )

    # ---- V3 storage with ones columns; q8 ----
    q8 = persist.tile([P, CB, N], FP8, tag="q8")
    v_aug = persist.tile([P, MB, C + 2], FP8, tag="v_aug")
    ones_c8 = persist.tile([P, 1], FP8, tag="ones_c8")
    nc.vector.memset(ones_c8, 1.0)
    nc.scalar.copy(out=v_aug[:, :, C : C + 2], in_=ones_c8.to_broadcast((P, MB, 2)))

    def q_chunk(ch):
        sl = slice(ch * NCHUNK, (ch + 1) * NCHUNK)
        pq = ps_sc.tile([P, 2 * NCHUNK], F32, tag="ps_sc", name=f"pq_{ch}")
        for cb in range(CB):
            qsl = slice(cb * NCHUNK, (cb + 1) * NCHUNK)
            nc.tensor.matmul(
                pq[:, qsl],
                lhsT=a8[:, :, cb * P : (cb + 1) * P],
                rhs=t8[:, :, sl],
                start=True,
                stop=False,
                perf_mode=DR,
            )
            nc.tensor.matmul(
                pq[:, qsl],
                lhsT=a8l[:, :, cb * P : (cb + 1) * P],
                rhs=t8[:, :, sl],
                start=False,
                stop=False,
                perf_mode=DR,
            )
            # + w_col x ones (the Wk^T bq bias column on q2)
            nc.tensor.matmul(
                pq[:, qsl],
                lhsT=w_row[0:1, cb * P : (cb + 1) * P],
                rhs=ones_nc,
                start=False,
                stop=True,
            )
        pqv = pq.rearrange("p (cb x) -> p cb x", cb=CB)
        if Q8_ENG == "A":
            nc.scalar.copy(out=q8[:, :, sl], in_=pqv)
        else:
            nc.vector.tensor_copy(out=q8[:, :, sl], in_=pqv)

    def v_pair(bp_, eng):
        vps = ps_sc.tile([P, 2 * NCHUNK], F32, tag="ps_sc", name=f"vb_{bp_}")
        for h in range(2):
            nb = 2 * bp_ + h
            vsl = slice(h * NCHUNK, h * NCHUNK + C)
            nc.tensor.matmul(
                vps[:, vsl],
                lhsT=t8[:, :, nb * P : (nb + 1) * P],
                rhs=wvp8[:, :, :],
                start=True,
                stop=False,
                perf_mode=DR,
            )
            nc.tensor.matmul(
                vps[:, vsl],
                lhsT=t8[:, :, nb * P : (nb + 1) * P],
                rhs=wvp8l[:, :, :],
                start=False,
                stop=False,
                perf_mode=DR,
            )
            # + ones x b' (residual+proj bias, recovered by the normalize fold)
            nc.tensor.matmul(
                vps[:, vsl], lhsT=ones_bf, rhs=b_row, start=False, stop=True
            )
        vv = vps.rearrange("p (h x) -> p h x", h=2)[:, :, 0:C]
        if eng == "A":
            nc.scalar.copy(out=v_aug[:, 2 * bp_ : 2 * bp_ + 2, 0:C], in_=vv)
        else:
            nc.vector.tensor_copy(out=v_aug[:, 2 * bp_ : 2 * bp_ + 2, 0:C], in_=vv)

    ex_const = None
    if exp_mode == "none":
        ex_const = persist.tile([P, 2, NCHUNK], FP8, tag="ex_const")
        nc.vector.memset(ex_const, 0.25)

    def produce_pair(j, bp_, with_v):
        sched = EXP0_SCHED if (j == 0 and EXP0_SCHED is not None) else EXP_SCHED
        if with_v:
            veng = ("D" if sched[bp_] == "A" else "A") if V_ENG == "opp" else V_ENG
            v_pair(bp_, veng)
        jsl = slice((j % NJ) * NCHUNK, (j % NJ + 1) * NCHUNK)
        ss = ps_sc.tile([P, 2 * NCHUNK], F32, tag="ps_sc", name=f"ss_{j}_{bp_}")
        for h in range(2):
            i_ = 2 * bp_ + h
            nc.tensor.matmul(
                ss[:, h * NCHUNK : (h + 1) * NCHUNK],
                lhsT=t8[:, :, i_ * P : (i_ + 1) * P],
                rhs=q8[:, :, jsl],
                start=True,
                stop=True,
                perf_mode=DR,
            )
        if exp_mode == "none":
            return ex_const
        ex = sexp.tile([P, 2, NCHUNK], FP8, tag="exp", name=f"ex_{j}_{bp_}")
        ssv = ss.rearrange("p (h x) -> p h x", h=2)
        eng = (sched[bp_] if exp_mode == "mix"
               else ("A" if exp_mode == "act" else "D"))
        if eng == "A":
            nc.scalar.activation(
                out=ex, in_=ssv, func=AF.Exp, scale=SCALE, bias=nshift_col
            )
        else:
            nc.vector.tensor_scalar(
                out=ex.bitcast(mybir.dt.uint8), in0=ssv,
                scalar1=EA8, scalar2=EB8, op0=OP.mult, op1=OP.add,
            )
        return ex

    def pv_mm(pv_ps, bp_, ex_pair, jj0):
        for i, pvt in enumerate(pv_ps):
            jj = jj0 + i
            nc.tensor.matmul(
                pvt,
                lhsT=ex_pair[:, :, jj * P : (jj + 1) * P],
                rhs=v_aug[:, 2 * bp_ : 2 * bp_ + 2, :],
                start=(bp_ == 0),
                stop=(bp_ == NPAIR - 1),
                perf_mode=DR,
            )

    def half_epi(j, pv_ps, jj0):
        anms = []
        for i, pvt in enumerate(pv_ps):
            jj = jj0 + i
            rec = stmp.tile([P, 1], F32, tag="rec", name=f"rec_{j}_{jj}")
            nc.vector.reciprocal(out=rec, in_=pvt[:, C : C + 1])
            anm = stmp.tile([P, C], F32, tag="anm", name=f"anm_{j}_{jj}")
            if ANM_SCHED[jj] == "A":
                nc.scalar.activation(
                    out=anm, in_=pvt[:, 0:C], func=AF.Copy, scale=rec
                )
            else:
                nc.vector.tensor_scalar(
                    out=anm, in0=pvt[:, 0:C], scalar1=rec, scalar2=None,
                    op0=OP.mult,
                )
            anms.append(anm)
        return anms

    def epi_head(j, anms01):
        jsl = slice((j % NJ) * NCHUNK, (j % NJ + 1) * NCHUNK)
        tpr = ps_sc.tile([P, 2 * NCHUNK], F32, tag="ps_sc", name=f"tprl_{j}")
        for co in range(CB):
            base = co * NCHUNK
            nc.tensor.matmul(
                tpr[:, base : base + NCHUNK],
                lhsT=diag_s[:, co, :],
                rhs=t_bf[:, co, jsl],
                start=True,
                stop=False,
            )
            for jj in range(2):
                nc.tensor.matmul(
                    tpr[:, base + jj * P : base + (jj + 1) * P],
                    lhsT=anms01[jj][:, co * P : (co + 1) * P],
                    rhs=ident,
                    is_transpose=True,
                    start=False,
                    stop=False,
                )
        return tpr

    def epi_finish(j, tpr, anms23):
        jsl = slice((j % NJ) * NCHUNK, (j % NJ + 1) * NCHUNK)
        for co in range(CB):
            base = co * NCHUNK
            for i, jj in enumerate((2, 3)):
                nc.tensor.matmul(
                    tpr[:, base + jj * P : base + (jj + 1) * P],
                    lhsT=anms23[i][:, co * P : (co + 1) * P],
                    rhs=ident,
                    is_transpose=True,
                    start=False,
                    stop=(jj == JJ - 1),
                )
        for co in range(CB):
            ob = sout.tile([P, NCHUNK], F32, tag="out", name=f"obl_{j}_{co}")
            tslice = tpr[:, co * NCHUNK : (co + 1) * NCHUNK]
            if OB_SCHED[co] == "A":
                nc.scalar.copy(out=ob, in_=tslice)
            else:
                nc.vector.tensor_copy(out=ob, in_=tslice)
            nc.sync.dma_start(out=out_d[co * P : (co + 1) * P, jsl], in_=ob)

    def epi_tail(j, anms):
        jsl = slice((j % NJ) * NCHUNK, (j % NJ + 1) * NCHUNK)
        # transposes for both channel blocks share one borrowed score region
        tpr = ps_sc.tile([P, 2 * NCHUNK], F32, tag="ps_sc", name=f"tpr_{j}")
        for co in range(CB):
            for jj in range(JJ):
                nc.tensor.transpose(
                    tpr[:, co * NCHUNK + jj * P : co * NCHUNK + (jj + 1) * P],
                    anms[jj][:, co * P : (co + 1) * P],
                    ident,
                )
        for co in range(CB):
            ob = sout.tile([P, NCHUNK], F32, tag="out", name=f"ob_{j}_{co}")
            s_col, _ = gn_cols[co]
            nc.vector.scalar_tensor_tensor(
                out=ob,
                in0=t_cm[:, co, jsl],
                scalar=s_col,
                in1=tpr[:, co * NCHUNK : (co + 1) * NCHUNK],
                op0=OP.mult,
                op1=OP.add,
            )
            nc.sync.dma_start(out=out_d[co * P : (co + 1) * P, jsl], in_=ob)

    def pv_tiles(j, jj0):
        return [
            ps_pv.tile([P, C + 2], F32, tag="ps_pv", name=f"pv_{j}_{jj0 + i}")
            for i in range(2)
        ]

    # ---- pipelined chunk loop ----
    # PV runs in two half-passes (jj0/jj1 then jj2/jj3) over the chunk's
    # persistent ex tiles, so only 2 PV psum banks are live at a time and the
    # score pipeline gets 3 two-bank regions. Pair production runs LA pairs
    # ahead, across chunk boundaries, so the exp engines never drain during
    # the PV half-1 + epilogue tail.
    total_chunks = NJ * att_reps
    total_pairs = total_chunks * NPAIR

    def produce_g(g):
        return produce_pair(g // NPAIR, g % NPAIR, g < NPAIR)

    q_chunk(0)
    exs = {}
    produced = 0
    pending_tail = None
    for j in range(total_chunks):
        base = j * NPAIR
        pv01 = pv_tiles(j, 0)
        for bp_ in range(NPAIR):
            # last chunk's transpose/stt tail, once its anms are long done
            if bp_ == TAIL_BP and pending_tail is not None:
                epi_tail(*pending_tail)
                pending_tail = None
            # q8 for chunk j+1 must be issued before the pair lookahead
            # crosses the chunk boundary (pairs base+16+ read it)
            if bp_ == QPOS and j + 1 < NJ:
                q_chunk(j + 1)
            while produced <= min(base + bp_ + LA, total_pairs - 1):
                exs[produced] = produce_g(produced)
                produced += 1
            pv_mm(pv01, bp_, exs[base + bp_], 0)
        anms01 = half_epi(j, pv01, 0)
        pv23 = pv_tiles(j, 2)
        nxt_lim = min(base + NPAIR - 1 + XLA, total_pairs - 1)
        for bp_ in range(NPAIR):
            # keep next-chunk score matmuls flowing between PV half-1 mms
            # so the exp engines never drain during the chunk tail
            if bp_ % 2 == 0 and produced <= nxt_lim:
                exs[produced] = produce_g(produced)
                produced += 1
            pv_mm(pv23, bp_, exs.pop(base + bp_), 2)
        while produced <= nxt_lim:
            exs[produced] = produce_g(produced)
            produced += 1
        if j == total_chunks - 1:
            # nothing overlaps the final tail: start the transpose region
            # (residual seed + jj0/jj1 transposes) while PV half-1 finishes
            tpr_last = epi_head(j, anms01)
            anms23 = half_epi(j, pv23, 2)
            epi_finish(j, tpr_last, anms23)
        else:
            anms23 = half_epi(j, pv23, 2)
            pending_tail = (j, anms01 + anms23)
    if pending_tail is not None:
        epi_tail(*pending_tail)


def kernel(x, gamma, beta, Wq, bq, Wk, bk, Wv, bv, Wp, bp):
    if "nc" not in _CACHE:
        _CACHE["nc"] = build_nc()
    nc = _CACHE["nc"]

    x = np.ascontiguousarray(np.asarray(x, dtype=np.float32)).reshape(B, C, N)
    common = {
        "gamma": np.asarray(gamma, np.float32),
        "beta": np.asarray(beta, np.float32),
        "Wq": np.asarray(Wq, np.float32),
        "bq": np.asarray(bq, np.float32),
        "Wk": np.asarray(Wk, np.float32),
        "bk": np.asarray(bk, np.float32),
        "Wv": np.asarray(Wv, np.float32),
        "bv": np.asarray(bv, np.float32),
        "Wp": np.asarray(Wp, np.float32),
        "bp": np.asarray(bp, np.float32),
    }
    in_maps = [{"x": x[b], **common} for b in range(B)]
    res = bass_utils.run_bass_kernel_spmd(nc, in_maps, core_ids=list(range(B)))
    out = np.stack([res.results[b]["out"] for b in range(B)])
    return out.reshape(B, C, H, W)    if not T8_INLINE:
        for cb in range(CB):
            s_col, b_col = gn_cols[cb]
            for k in range(N // 1024):
                asl = slice(k * 1024, (k + 1) * 1024)
                eng = T8_SCHED[cb][k]
                if eng == "D":
                    nc.vector.tensor_scalar(
                        out=t8[:, cb, asl], in0=t_cm[:, cb, asl], scalar1=s_col,
                        scalar2=b_col, op0=OP.mult, op1=OP.add,
                    )
                elif eng == "P":
                    nc.gpsimd.tensor_scalar(
                        out=t8[:, cb, asl], in0=t_cm[:, cb, asl], scalar1=s_col,
                        scalar2=b_col, op0=OP.mult, op1=OP.add,
                    )
                else:
                    nc.scalar.activation(
                        out=t8[:, cb, asl], in_=t_cm[:, cb, asl], func=AF.Identity,
                        bias=b_col, scale=s_col,
                    )




# revision 55
# speedup vs baseline: 1.0963x; 1.0273x over previous
"""Trainium2 Bass kernel for an AttentionBlock (GroupNorm + single-head
full N^2 attention + output projection + residual), data-parallel over
batch: 8 samples on 8 NeuronCores, no collectives.

Shapes (hardcoded): x [8, 256, 64, 64]; weights [256, 256]; biases [256].
Per core: one batch sample, x viewed as [C=256, N=4096] channel-major.

Per-core pipeline (fp8 DoubleRow matmuls: 2 k-tiles of 128 contracted per
instruction -> full C=256 contraction per matmul at double fp8 rate):
  0. x loads as bf16 via casting gpsimd/SWDGE DMAs, halving the 4MB
     x traffic on the serial DMA path that gates the setup phase. The
     bf16 rounding is invisible downstream: GN stats average it out over
     131k elements/group, tokens are rounded to fp8 anyway, and the
     residual already used a bf16 copy.
  1. GroupNorm (8 groups) in C-major layout: per-partition bn_stats,
     cross-partition group reduction via tiny constant matmuls, applied as
     per-partition scale/bias. Rounded tokens t8 (fp8e4) feed all matmuls;
     computed in 1024-col ops spread over DVE/ACT/GPSIMD. The biases fold
     into matmuls: b' = b_gn + bp + Wp bv rides the V3 psum and
     w = Wk^T bq rides the q2 psum, each as a ones x row outer product.
  2. Wq/Wk fold: scores = t A t^T with A = Wq^T Wk (computed on-chip,
     stored dual fp8 hi+lo). The output projection folds into V:
     v3 = t (Wp Wv)^T (dual fp8), with appended ones columns so PV also
     produces softmax denominators.
  3. Attention over 512-query chunks, transposed: S^T = t8^T q8 with keys
     on partitions. Scores for a key-block PAIR live in ONE 2-bank psum
     region [P, 1024]; 3 regions rotate so a single 1024-col exp op per
     pair keeps both exp engines saturated (pair production runs LA=3
     ahead, and XLA=6 pairs of the next chunk are primed before each
     epilogue). exp alternates ACT (native Exp -> fp8) and DVE via a
     single-op Schraudolph: fp8e4 bits of exp(s*SCALE - SHIFT) equal
     sat_u8_rne(s*EA8 + EB8), and the DVE f32->u8 convert saturates on
     HW (verified), so one tensor_scalar yields PV-ready fp8 weights.
     The global SHIFT keeps exp within fp8e4 range (max 240) and cancels
     in the normalization.
  4. PV runs in two half-passes (query sub-blocks jj0/jj1 then jj2/jj3)
     over the chunk's persistent SBUF ex tiles, so only 2 PV psum banks
     are live and the score pipeline gets its 3rd region. Epilogue per
     chunk: normalize by 1/colsum per 128-query block, then a borrowed
     score region is seeded with the scaled residual s*x via a diag(s)
     bf16 matmul (reading the bf16 x directly) and the
     TensorE transposes accumulate the C-major projection on top; the
     psum -> sbuf move is then a plain engine-assignable copy, DMA out.
     The transpose/copy tail of chunk j is deferred into chunk j+1 so it
     overlaps pair production; only the last chunk runs it eagerly, split
     around its PV half-1 pass.
"""
import numpy as np

import concourse.bacc as bacc
import concourse.mybir as mybir
import concourse.tile as tile
from concourse import bass_utils

F32 = mybir.dt.float32
F32R = mybir.dt.float32r
BF16 = mybir.dt.bfloat16
FP8 = mybir.dt.float8e4
I32 = mybir.dt.int32
AF = mybir.ActivationFunctionType
OP = mybir.AluOpType
DR = mybir.MatmulPerfMode.DoubleRow

B = 8
C = 256
H = 64
W = 64
N = H * W  # 4096 tokens
G = 8  # groups
GS = C // G  # 32 channels per group
P = 128
CB = C // P  # 2 channel blocks
EPS = 1e-5
NCHUNK = 512  # query chunk (matmul moving free dim)
NJ = N // NCHUNK  # 8
MB = N // P  # 32 key blocks
NPAIR = MB // 2  # 16 key-block pairs
JJ = NCHUNK // P  # 4 query sub-blocks per chunk
SCALE = C ** (-0.5)
SHIFT = 3.5  # exp(s*SCALE - SHIFT): keeps fp8e4 range, cancels in softmax

# Schraudolph exp-as-uint8-fp8-bits constants:
#   fp8e4_bits(exp(y)) ~= sat_u8_rne(y*8*log2e + 56), y = s*SCALE - SHIFT.
# DVE f32->u8 conversion saturates on HW (verified: neg -> 0, >255 -> 255,
# RNE), so one tensor_scalar per pair region produces PV-ready fp8 weights.
LOG2E = 1.4426950408889634
EA8 = float(SCALE * 8.0 * LOG2E)
EB8 = float(56.0 - SHIFT * 8.0 * LOG2E)

# Engine split for the 16 merged exp pair-ops per chunk: "A" = ACT native
# exp, "D" = DVE single-op uint8 Schraudolph. Strict alternation keeps the
# 3-region score pipeline symmetric (each engine fires every other pair).
EXP_SCHED = ["A", "D"] * 8
# Normalize (anm) engine per query sub-block.
ANM_SCHED = ["A", "D", "A", "D"]
Q8_ENG = "A"
XLA = 6  # pairs primed into the next chunk before the epilogue ops
LA = 3  # in-chunk pair lookahead (= score region count)
QPOS = 6  # pair index in chunk j where q8 for chunk j+1 is produced
TAIL_BP = 0  # where in the next chunk the deferred transpose/stt tail runs
# t8 1024-col slices: [cb][k] engine, "P" = GPSIMD (otherwise idle).
T8_SCHED = [["D", "A", "P", "D"], ["A", "D", "P", "A"]]
T8_INLINE = False  # emit each cb's t8 inside the GN loop
DMA_MODE = "orig"  # "orig" | "wfirst_fine"
STATS_MODE = "bn"  # "bn" | "split": cb1 stats via ACT/DVE accum_out ops
GN_HIPRI = False  # schedule the GN stats chain at max priority
SOUT_BUFS = 6
STMP_BUFS = 8
EXP0_SCHED = None  # chunk-0 exp split override (None -> EXP_SCHED)
FRONT_PIN = 0  # 0=off; else scale for wait_until pins on the prep blocks
V_ENG = "opp"  # chunk-0 v_aug copy engine: "opp" (opposite of exp) | "A" | "D"

_CACHE: dict = {}


def build_nc(att_reps=1, exp_mode="mix"):
    """exp_mode: "mix" (EXP_SCHED), "act", "dve" (timing calibration),
    "none" (skip exp: PV reads a constant tile; output garbage)."""
    nc = bacc.Bacc(
        "TRN2",
        target_bir_lowering=False,
        debug=False,
        enable_asserts=False,
        num_devices=B,
    )

    x_d = nc.dram_tensor("x", [C, N], F32, kind="ExternalInput")
    gamma_d = nc.dram_tensor("gamma", [C], F32, kind="ExternalInput")
    beta_d = nc.dram_tensor("beta", [C], F32, kind="ExternalInput")
    w_d = {}
    b_d = {}
    for nm in ("q", "k", "v", "p"):
        w_d[nm] = nc.dram_tensor(f"W{nm}", [C, C], F32, kind="ExternalInput")
        b_d[nm] = nc.dram_tensor(f"b{nm}", [C], F32, kind="ExternalInput")
    out_d = nc.dram_tensor("out", [C, N], F32, kind="ExternalOutput")

    ident_d = nc.inline_tensor(np.eye(P, dtype=np.float32), name="ident")
    # Group-sum selector: [P, G/CB] with 1/GS entries -> group means directly.
    gsum_np = np.zeros((P, G // CB), np.float32)
    for p in range(P):
        gsum_np[p, p // GS] = 1.0 / GS
    gsum_d = nc.inline_tensor(gsum_np, name="gsum")
    # Group-broadcast selector: [G/CB, P] with 1s.
    gbc_np = np.zeros((G // CB, P), np.float32)
    for p in range(P):
        gbc_np[p // GS, p] = 1.0
    gbc_d = nc.inline_tensor(gbc_np, name="gbc")

    from contextlib import ExitStack

    with tile.TileContext(nc) as tc:
        with ExitStack() as ctx:
            _build_tile(
                ctx, tc, x_d, gamma_d, beta_d, w_d, b_d, out_d, ident_d, gsum_d,
                gbc_d, att_reps, exp_mode,
            )
    nc.compile()
    return nc


def _build_tile(ctx, tc, x_d, gamma_d, beta_d, w_d, b_d, out_d, ident_d, gsum_d, gbc_d, att_reps=1, exp_mode="mix"):
    nc = tc.nc

    persist = ctx.enter_context(tc.tile_pool(name="persist", bufs=1))
    staging = ctx.enter_context(tc.tile_pool(name="staging", bufs=4))
    # ex tiles live for a whole chunk (both PV half-passes) + lookahead
    sexp = ctx.enter_context(tc.tile_pool(name="sexp", bufs=NPAIR + XLA + 1))
    sout = ctx.enter_context(tc.tile_pool(name="sout", bufs=SOUT_BUFS))
    stmp = ctx.enter_context(tc.tile_pool(name="stmp", bufs=STMP_BUFS))
    # ps_sc: [P, 1024] f32 2-bank score/psum regions (bufs=3 -> 6 banks).
    # Also borrowed for q-projection psum and the epilogue transpose region.
    ps_sc = ctx.enter_context(tc.tile_pool(name="ps_sc", bufs=3, space="PSUM"))
    # ps_pv: PV accumulators, 2 live at a time (PV runs in two half-passes
    # over the persistent SBUF ex tiles: jj0/jj1 then jj2/jj3)
    ps_pv = ctx.enter_context(tc.tile_pool(name="ps_pv", bufs=2, space="PSUM"))

    t_cm = persist.tile([P, CB, N], BF16, tag="t_cm")  # raw x, bf16, C-major
    t_bf = t_cm  # epilogue residual reads the same bf16 tensor
    t8 = persist.tile([P, CB, N], FP8, tag="t8")  # groupnormed tokens, fp8
    NSUB = N // 512  # bn_stats free-dim limit

    # ---- x load in 1024-col slices over 4 DMA queues (2 each, cb0 first so
    # bn_stats can start early); GN constants (gsum/gbc) lead the scalar queue
    if not FRONT_PIN:
        gsum = persist.tile([P, G // CB], F32, tag="gsum")
        nc.scalar.dma_start(out=gsum, in_=gsum_d[:, :])
        gbc = persist.tile([G // CB, P], F32, tag="gbc")
        nc.scalar.dma_start(out=gbc, in_=gbc_d[:, :])

    # x slices: sync s0/s3/s6, scalar s1/s4/s7, gpsimd s2/s5. Weights ride
    # the fast sync/HWDGE queue interleaved with x so A/wvp8 prep can start
    # ~5us in (the gpsimd/SWDGE queue generates descriptors on the Q7 and is
    # much slower per DMA).
    XS = 1024
    w_stage = {
        nm: staging.tile([P, CB, C], F32, tag="w_stage", name=f"w_sb_{nm}")
        for nm in ("q", "k", "v", "p")
    }

    def x_load(eng, s_):
        # f32 -> bf16 cast in flight: casting DMAs are SWDGE/gpsimd-only
        cb, i_ = divmod(s_, 4)
        sl = slice(i_ * XS, (i_ + 1) * XS)
        nc.gpsimd.dma_start(out=t_cm[:, cb, sl], in_=x_d[cb * P : (cb + 1) * P, sl])

    def w_load(nm):
        nc.sync.dma_start(
            out=w_stage[nm], in_=w_d[nm][:, :].rearrange("(b p) i -> p b i", p=P)
        )

    if DMA_MODE == "wfirst_fine":
        # ident+Wq+Wk first (unblock A-prep), x in 512-col slices cb0-first,
        # Wv/Wp after (V path isn't needed until ~18us)
        ident = persist.tile([P, P], F32, tag="ident")
        nc.sync.dma_start(out=ident, in_=ident_d[:, :])
        w_load("q")
        w_load("k")
        xq3 = [nc.sync, nc.scalar, nc.gpsimd]
        for h_ in range(16):
            cb, i_ = divmod(h_, 8)
            sl = slice(i_ * 512, (i_ + 1) * 512)
            xq3[h_ % 3].dma_start(
                out=t_cm[:, cb, sl], in_=x_d[cb * P : (cb + 1) * P, sl]
            )
        w_load("v")
        w_load("p")
    elif DMA_MODE == "worder2":
        # ident+Wq+Wk lead the serial DMA device (~1.8us) so the cold fp32
        # A-prep matmul chain (~5us on PE) runs during the x load instead of
        # after it; Wv/Wp follow x (the V path isn't needed until ~18us).
        ident = persist.tile([P, P], F32, tag="ident")
        nc.sync.dma_start(out=ident, in_=ident_d[:, :])
        for nm, eng in (("q", nc.sync), ("k", nc.sync)):
            eng.dma_start(
                out=w_stage[nm], in_=w_d[nm][:, :].rearrange("(b p) i -> p b i", p=P)
            )
        for s_ in range(8):
            x_load([nc.sync, nc.scalar, nc.gpsimd][s_ % 3], s_)
        for nm, eng in (("v", nc.sync), ("p", nc.sync)):
            eng.dma_start(
                out=w_stage[nm], in_=w_d[nm][:, :].rearrange("(b p) i -> p b i", p=P)
            )
    elif DMA_MODE == "fine":
        xq3 = [nc.sync, nc.scalar, nc.gpsimd]
        for h_ in range(16):
            cb, i_ = divmod(h_, 8)
            sl = slice(i_ * 512, (i_ + 1) * 512)
            xq3[h_ % 3].dma_start(
                out=t_cm[:, cb, sl], in_=x_d[cb * P : (cb + 1) * P, sl]
            )
        for nm, eng in (("q", nc.gpsimd), ("k", nc.gpsimd), ("v", nc.sync), ("p", nc.sync)):
            eng.dma_start(
                out=w_stage[nm], in_=w_d[nm][:, :].rearrange("(b p) i -> p b i", p=P)
            )
    else:
        for s_ in range(8):
            x_load([nc.sync, nc.scalar, nc.gpsimd][s_ % 3], s_)
        for nm, eng in (("q", nc.sync), ("k", nc.sync), ("v", nc.sync), ("p", nc.sync)):
            eng.dma_start(
                out=w_stage[nm], in_=w_d[nm][:, :].rearrange("(b p) i -> p b i", p=P)
            )

    def col_tile(dram_vec, tag, eng):
        t = persist.tile([P, CB], F32, tag=tag)
        eng.dma_start(out=t, in_=dram_vec[:].rearrange("(b p) -> p b", p=P))
        return t

    if FRONT_PIN:
        # behind the x slices on the scalar queue so x1 isn't pushed to the
        # back of the serial DMA device (bn_stats #3 stall)
        gsum = persist.tile([P, G // CB], F32, tag="gsum")
        nc.scalar.dma_start(out=gsum, in_=gsum_d[:, :])
        gbc = persist.tile([G // CB, P], F32, tag="gbc")
        nc.scalar.dma_start(out=gbc, in_=gbc_d[:, :])
    gamma_col = col_tile(gamma_d, "gamma_col", nc.scalar)
    beta_col = col_tile(beta_d, "beta_col", nc.scalar)
    if DMA_MODE not in ("wfirst_fine", "worder2"):
        ident = persist.tile([P, P], F32, tag="ident")
        nc.gpsimd.dma_start(out=ident, in_=ident_d[:, :])
    bq_col = col_tile(b_d["q"], "bq_col", nc.gpsimd)
    bv_col = col_tile(b_d["v"], "bv_col", nc.gpsimd)
    bp_col = col_tile(b_d["p"], "bp_col", nc.gpsimd)

    # ---- A = Wq^T Wk  [c, c'] as dual fp8 (hi + residual lo) ----
    a8 = persist.tile([P, CB, C], FP8, tag="a8")
    a8l = persist.tile([P, CB, C], FP8, tag="a8l")
    for cb in range(CB):
        aps = ps_sc.tile([P, 2 * NCHUNK], F32, tag="ps_sc", name=f"aps_{cb}")
        for mb in range(CB):
            nc.tensor.matmul(
                aps[:, 0:C],
                lhsT=w_stage["q"][:, mb, cb * P : (cb + 1) * P],
                rhs=w_stage["k"][:, mb, :],
                start=(mb == 0),
                stop=(mb == CB - 1),
            )
        nc.scalar.copy(out=a8[:, cb, :], in_=aps[:, 0:C])
        nc.vector.tensor_tensor(
            out=a8l[:, cb, :], in0=aps[:, 0:C], in1=a8[:, cb, :], op=OP.subtract
        )

    # ---- GroupNorm stats -> per-channel scale s_col, bias b_col ----
    gn_cols = []
    from contextlib import nullcontext
    with (tc.high_priority() if GN_HIPRI else nullcontext()):
      for cb in range(CB):
        xt = t_cm[:, cb, :]
        if STATS_MODE == "split" and cb == 1:
            # cb1 stats off the DVE serial path: mean partials via ACT
            # Identity+accum (1/N folded into scale), E[x^2] partials via DVE
            # scalar_tensor_tensor+accum ((x/N) * x)
            sacc = stmp.tile([P, 8], F32, tag="gn_acc")
            scr_a = stmp.tile([P, 1024], F32, tag="gn_scr_a", bufs=1)
            scr_d = stmp.tile([P, 1024], F32, tag="gn_scr_d", bufs=1)
            for k4 in range(4):
                asl = slice(k4 * 1024, (k4 + 1) * 1024)
                nc.scalar.activation(
                    out=scr_a, in_=xt[:, asl], func=AF.Identity,
                    scale=1.0 / N, accum_out=sacc[:, k4 : k4 + 1],
                )
                nc.vector.scalar_tensor_tensor(
                    out=scr_d, in0=xt[:, asl], scalar=1.0 / N, in1=xt[:, asl],
                    op0=OP.mult, op1=OP.mult,
                    accum_out=sacc[:, 4 + k4 : 5 + k4],
                )
            stats2 = stmp.tile([P, 2], F32, tag="gn_stats2")
            h1 = stmp.tile([P, 2], F32, tag="gn_h1")
            nc.vector.tensor_add(out=h1[:, 0:1], in0=sacc[:, 0:1], in1=sacc[:, 1:2])
            nc.vector.tensor_add(out=h1[:, 1:2], in0=sacc[:, 2:3], in1=sacc[:, 3:4])
            nc.vector.tensor_add(out=stats2[:, 0:1], in0=h1[:, 0:1], in1=h1[:, 1:2])
            h2 = stmp.tile([P, 2], F32, tag="gn_h2")
            nc.vector.tensor_add(out=h2[:, 0:1], in0=sacc[:, 4:5], in1=sacc[:, 5:6])
            nc.vector.tensor_add(out=h2[:, 1:2], in0=sacc[:, 6:7], in1=sacc[:, 7:8])
            nc.vector.tensor_add(out=stats2[:, 1:2], in0=h2[:, 0:1], in1=h2[:, 1:2])
        else:
            stats = stmp.tile([P, NSUB, 6], F32, tag="gn_stats")
            for s in range(NSUB):
                nc.vector.bn_stats(out=stats[:, s, :], in_=xt[:, s * 512 : (s + 1) * 512])
            mv = stmp.tile([P, 2], F32, tag="gn_mv")
            nc.vector.bn_aggr(out=mv, in_=stats)
            # stats2 = (mean_p, E[x^2]_p)
            stats2 = stmp.tile([P, 2], F32, tag="gn_stats2")
            nc.vector.tensor_copy(out=stats2[:, 0:1], in_=mv[:, 0:1])
            nc.vector.tensor_tensor(
                out=stats2[:, 1:2], in0=mv[:, 0:1], in1=mv[:, 0:1], op=OP.mult
            )
            nc.vector.tensor_add(out=stats2[:, 1:2], in0=stats2[:, 1:2], in1=mv[:, 1:2])
        # group reduce: [G/CB, 2] = gsum.T @ stats2  (means already /GS)
        gps = ps_sc.tile([P, 2 * NCHUNK], F32, tag="ps_sc", name=f"gps_{cb}")
        nc.tensor.matmul(gps[0 : G // CB, 0:2], lhsT=gsum, rhs=stats2, start=True, stop=True)
        # rstd_g = 1/sqrt(E2_g - mean_g^2 + eps)
        gsb = stmp.tile([G // CB, 2], F32, tag="gn_gsb")
        nc.vector.tensor_copy(out=gsb, in_=gps[0 : G // CB, 0:2])
        gpack = stmp.tile([G // CB, 2], F32, tag="gn_gpack")
        nc.vector.tensor_copy(out=gpack[:, 0:1], in_=gsb[:, 0:1])
        gvar = stmp.tile([G // CB, 1], F32, tag="gn_gvar")
        nc.vector.tensor_tensor(
            out=gvar, in0=gsb[:, 0:1], in1=gsb[:, 0:1], op=OP.mult
        )
        nc.vector.tensor_tensor(
            out=gvar, in0=gsb[:, 1:2], in1=gvar, op=OP.subtract
        )
        # rstd = 1/sqrt(var+eps) (DVE pow is rejected by the ISA checker)
        eps_t = stmp.tile([G // CB, 1], F32, tag="gn_eps")
        nc.vector.memset(eps_t, EPS)
        nc.scalar.activation(out=gvar, in_=gvar, func=AF.Sqrt, bias=eps_t)
        nc.vector.reciprocal(out=gpack[:, 1:2], in_=gvar)
        # broadcast to channels: [P, 2] = gbc.T @ gpack
        bps = ps_sc.tile([P, 2 * NCHUNK], F32, tag="ps_sc", name=f"bps_{cb}")
        nc.tensor.matmul(bps[:, 0:2], lhsT=gbc, rhs=gpack, start=True, stop=True)
        # s_col = rstd_c * gamma_c ; b_col = beta_c - mean_c * s_col
        s_col = stmp.tile([P, 1], F32, tag="gn_scol")
        nc.vector.tensor_tensor(
            out=s_col, in0=bps[:, 1:2], in1=gamma_col[:, cb : cb + 1], op=OP.mult
        )
        b_col = stmp.tile([P, 1], F32, tag="gn_bcol")
        nc.vector.tensor_tensor(out=b_col, in0=bps[:, 0:1], in1=s_col, op=OP.mult)
        nc.vector.tensor_tensor(
            out=b_col, in0=beta_col[:, cb : cb + 1], in1=b_col, op=OP.subtract
        )
        gn_cols.append((s_col, b_col))
        if not T8_INLINE:
            continue
        # rounded fp8 tokens for this channel block immediately (cb0's t8
        # overlaps cb1's stats): 1024-col ops over DVE/ACT/GPSIMD
        for k in range(N // 1024):
            asl = slice(k * 1024, (k + 1) * 1024)
            eng = T8_SCHED[cb][k]
            if eng == "D":
                nc.vector.tensor_scalar(
                    out=t8[:, cb, asl], in0=t_cm[:, cb, asl], scalar1=s_col,
                    scalar2=b_col, op0=OP.mult, op1=OP.add,
                )
            elif eng == "P":
                nc.gpsimd.tensor_scalar(
                    out=t8[:, cb, asl], in0=t_cm[:, cb, asl], scalar1=s_col,
                    scalar2=b_col, op0=OP.mult, op1=OP.add,
                )
            else:
                nc.scalar.activation(
                    out=t8[:, cb, asl], in_=t_cm[:, cb, asl], func=AF.Identity,
                    bias=b_col, scale=s_col,
                )

    # ---- V3 storage with ones columns; q8 ----
    q8 = persist.tile([P, CB, N], FP8, tag="q8")
    v_aug = persist.tile([P, MB, C + 2], FP8, tag="v_aug")
    ones_c8 = persist.tile([P, 1], FP8, tag="ones_c8")
    nc.vector.memset(ones_c8, 1.0)
    nc.scalar.copy(out=v_aug[:, :, C : C + 2], in_=ones_c8.to_broadcast((P, MB, 2)))

    def q_chunk(ch):
        sl = slice(ch * NCHUNK, (ch + 1) * NCHUNK)
        pq = ps_sc.tile([P, 2 * NCHUNK], F32, tag="ps_sc", name=f"pq_{ch}")
        for cb in range(CB):
            qsl = slice(cb * NCHUNK, (cb + 1) * NCHUNK)
            nc.tensor.matmul(
                pq[:, qsl],
                lhsT=a8[:, :, cb * P : (cb + 1) * P],
                rhs=t8[:, :, sl],
                start=True,
                stop=False,
                perf_mode=DR,
            )
            nc.tensor.matmul(
                pq[:, qsl],
                lhsT=a8l[:, :, cb * P : (cb + 1) * P],
                rhs=t8[:, :, sl],
                start=False,
                stop=False,
                perf_mode=DR,
            )
            # + w_col x ones (the Wk^T bq bias column on q2)
            nc.tensor.matmul(
                pq[:, qsl],
                lhsT=w_row[0:1, cb * P : (cb + 1) * P],
                rhs=ones_nc,
                start=False,
                stop=True,
            )
        pqv = pq.rearrange("p (cb x) -> p cb x", cb=CB)
        if Q8_ENG == "A":
            nc.scalar.copy(out=q8[:, :, sl], in_=pqv)
        else:
            nc.vector.tensor_copy(out=q8[:, :, sl], in_=pqv)

    def v_pair(bp_, eng):
        vps = ps_sc.tile([P, 2 * NCHUNK], F32, tag="ps_sc", name=f"vb_{bp_}")
        for h in range(2):
            nb = 2 * bp_ + h
            vsl = slice(h * NCHUNK, h * NCHUNK + C)
            nc.tensor.matmul(
                vps[:, vsl],
                lhsT=t8[:, :, nb * P : (nb + 1) * P],
                rhs=wvp8[:, :, :],
                start=True,
                stop=False,
                perf_mode=DR,
            )
            nc.tensor.matmul(
                vps[:, vsl],
                lhsT=t8[:, :, nb * P : (nb + 1) * P],
                rhs=wvp8l[:, :, :],
                start=False,
                stop=False,
                perf_mode=DR,
            )
            # + ones x b' (residual+proj bias, recovered by the normalize fold)
            nc.tensor.matmul(
                vps[:, vsl], lhsT=ones_bf, rhs=b_row, start=False, stop=True
            )
        vv = vps.rearrange("p (h x) -> p h x", h=2)[:, :, 0:C]
        if eng == "A":
            nc.scalar.copy(out=v_aug[:, 2 * bp_ : 2 * bp_ + 2, 0:C], in_=vv)
        else:
            nc.vector.tensor_copy(out=v_aug[:, 2 * bp_ : 2 * bp_ + 2, 0:C], in_=vv)

    ex_const = None
    if exp_mode == "none":
        ex_const = persist.tile([P, 2, NCHUNK], FP8, tag="ex_const")
        nc.vector.memset(ex_const, 0.25)

    def produce_pair(j, bp_, with_v):
        sched = EXP0_SCHED if (j == 0 and EXP0_SCHED is not None) else EXP_SCHED
        if with_v:
            veng = ("D" if sched[bp_] == "A" else "A") if V_ENG == "opp" else V_ENG
            v_pair(bp_, veng)
        jsl = slice((j % NJ) * NCHUNK, (j % NJ + 1) * NCHUNK)
        ss = ps_sc.tile([P, 2 * NCHUNK], F32, tag="ps_sc", name=f"ss_{j}_{bp_}")
        for h in range(2):
            i_ = 2 * bp_ + h
            nc.tensor.matmul(
                ss[:, h * NCHUNK : (h + 1) * NCHUNK],
                lhsT=t8[:, :, i_ * P : (i_ + 1) * P],
                rhs=q8[:, :, jsl],
                start=True,
                stop=True,
                perf_mode=DR,
            )
        if exp_mode == "none":
            return ex_const
        ex = sexp.tile([P, 2, NCHUNK], FP8, tag="exp", name=f"ex_{j}_{bp_}")
        ssv = ss.rearrange("p (h x) -> p h x", h=2)
        eng = (sched[bp_] if exp_mode == "mix"
               else ("A" if exp_mode == "act" else "D"))
        if eng == "A":
            nc.scalar.activation(
                out=ex, in_=ssv, func=AF.Exp, scale=SCALE, bias=nshift_col
            )
        else:
            nc.vector.tensor_scalar(
                out=ex.bitcast(mybir.dt.uint8), in0=ssv,
                scalar1=EA8, scalar2=EB8, op0=OP.mult, op1=OP.add,
            )
        return ex

    def pv_mm(pv_ps, bp_, ex_pair, jj0):
        for i, pvt in enumerate(pv_ps):
            jj = jj0 + i
            nc.tensor.matmul(
                pvt,
                lhsT=ex_pair[:, :, jj * P : (jj + 1) * P],
                rhs=v_aug[:, 2 * bp_ : 2 * bp_ + 2, :],
                start=(bp_ == 0),
                stop=(bp_ == NPAIR - 1),
                perf_mode=DR,
            )

    def half_epi(j, pv_ps, jj0):
        anms = []
        for i, pvt in enumerate(pv_ps):
            jj = jj0 + i
            rec = stmp.tile([P, 1], F32, tag="rec", name=f"rec_{j}_{jj}")
            nc.vector.reciprocal(out=rec, in_=pvt[:, C : C + 1])
            anm = stmp.tile([P, C], F32, tag="anm", name=f"anm_{j}_{jj}")
            if ANM_SCHED[jj] == "A":
                nc.scalar.activation(
                    out=anm, in_=pvt[:, 0:C], func=AF.Copy, scale=rec
                )
            else:
                nc.vector.tensor_scalar(
                    out=anm, in0=pvt[:, 0:C], scalar1=rec, scalar2=None,
                    op0=OP.mult,
                )
            anms.append(anm)
        return anms

    def epi_head(j, anms01):
        jsl = slice((j % NJ) * NCHUNK, (j % NJ + 1) * NCHUNK)
        tpr = ps_sc.tile([P, 2 * NCHUNK], F32, tag="ps_sc", name=f"tprl_{j}")
        for co in range(CB):
            base = co * NCHUNK
            nc.tensor.matmul(
                tpr[:, base : base + NCHUNK],
                lhsT=diag_s[:, co, :],
                rhs=t_bf[:, co, jsl],
                start=True,
                stop=False,
            )
            for jj in range(2):
                nc.tensor.matmul(
                    tpr[:, base + jj * P : base + (jj + 1) * P],
                    lhsT=anms01[jj][:, co * P : (co + 1) * P],
                    rhs=ident,
                    is_transpose=True,
                    start=False,
                    stop=False,
                )
        return tpr

    def epi_finish(j, tpr, anms23):
        jsl = slice((j % NJ) * NCHUNK, (j % NJ + 1) * NCHUNK)
        for co in range(CB):
            base = co * NCHUNK
            for i, jj in enumerate((2, 3)):
                nc.tensor.matmul(
                    tpr[:, base + jj * P : base + (jj + 1) * P],
                    lhsT=anms23[i][:, co * P : (co + 1) * P],
                    rhs=ident,
                    is_transpose=True,
                    start=False,
                    stop=(jj == JJ - 1),
                )
        for co in range(CB):
            ob = sout.tile([P, NCHUNK], F32, tag="out", name=f"obl_{j}_{co}")
            tslice = tpr[:, co * NCHUNK : (co + 1) * NCHUNK]
            if OB_SCHED[co] == "A":
                nc.scalar.copy(out=ob, in_=tslice)
            else:
                nc.vector.tensor_copy(out=ob, in_=tslice)
            nc.sync.dma_start(out=out_d[co * P : (co + 1) * P, jsl], in_=ob)

    def epi_tail(j, anms):
        jsl = slice((j % NJ) * NCHUNK, (j % NJ + 1) * NCHUNK)
        # transposes for both channel blocks share one borrowed score region
        tpr = ps_sc.tile([P, 2 * NCHUNK], F32, tag="ps_sc", name=f"tpr_{j}")
        for co in range(CB):
            for jj in range(JJ):
                nc.tensor.transpose(
                    tpr[:, co * NCHUNK + jj * P : co * NCHUNK + (jj + 1) * P],
                    anms[jj][:, co * P : (co + 1) * P],
                    ident,
                )
        for co in range(CB):
            ob = sout.tile([P, NCHUNK], F32, tag="out", name=f"ob_{j}_{co}")
            s_col, _ = gn_cols[co]
            nc.vector.scalar_tensor_tensor(
                out=ob,
                in0=t_cm[:, co, jsl],
                scalar=s_col,
                in1=tpr[:, co * NCHUNK : (co + 1) * NCHUNK],
                op0=OP.mult,
                op1=OP.add,
            )
            nc.sync.dma_start(out=out_d[co * P : (co + 1) * P, jsl], in_=ob)

    def pv_tiles(j, jj0):
        return [
            ps_pv.tile([P, C + 2], F32, tag="ps_pv", name=f"pv_{j}_{jj0 + i}")
            for i in range(2)
        ]

    # ---- pipelined chunk loop ----
    # PV runs in two half-passes (jj0/jj1 then jj2/jj3) over the chunk's
    # persistent ex tiles, so only 2 PV psum banks are live at a time and the
    # score pipeline gets 3 two-bank regions. Pair production runs LA pairs
    # ahead, across chunk boundaries, so the exp engines never drain during
    # the PV half-1 + epilogue tail.
    total_chunks = NJ * att_reps
    total_pairs = total_chunks * NPAIR

    def produce_g(g):
        return produce_pair(g // NPAIR, g % NPAIR, g < NPAIR)

    q_chunk(0)
    exs = {}
    produced = 0
    pending_tail = None
    for j in range(total_chunks):
        base = j * NPAIR
        pv01 = pv_tiles(j, 0)
        for bp_ in range(NPAIR):
            # last chunk's transpose/stt tail, once its anms are long done
            if bp_ == TAIL_BP and pending_tail is not None:
                epi_tail(*pending_tail)
                pending_tail = None
            # q8 for chunk j+1 must be issued before the pair lookahead
            # crosses the chunk boundary (pairs base+16+ read it)
            if bp_ == QPOS and j + 1 < NJ:
                q_chunk(j + 1)
            while produced <= min(base + bp_ + LA, total_pairs - 1):
                exs[produced] = produce_g(produced)
                produced += 1
            pv_mm(pv01, bp_, exs[base + bp_], 0)
        anms01 = half_epi(j, pv01, 0)
        pv23 = pv_tiles(j, 2)
        nxt_lim = min(base + NPAIR - 1 + XLA, total_pairs - 1)
        for bp_ in range(NPAIR):
            # keep next-chunk score matmuls flowing between PV half-1 mms
            # so the exp engines never drain during the chunk tail
            if bp_ % 2 == 0 and produced <= nxt_lim:
                exs[produced] = produce_g(produced)
                produced += 1
            pv_mm(pv23, bp_, exs.pop(base + bp_), 2)
        while produced <= nxt_lim:
            exs[produced] = produce_g(produced)
            produced += 1
        if j == total_chunks - 1:
            # nothing overlaps the final tail: start the transpose region
            # (residual seed + jj0/jj1 transposes) while PV half-1 finishes
            tpr_last = epi_head(j, anms01)
            anms23 = half_epi(j, pv23, 2)
            epi_finish(j, tpr_last, anms23)
        else:
            anms23 = half_epi(j, pv23, 2)
            pending_tail = (j, anms01 + anms23)
    if pending_tail is not None:
        epi_tail(*pending_tail)


def kernel(x, gamma, beta, Wq, bq, Wk, bk, Wv, bv, Wp, bp):
    if "nc" not in _CACHE:
        _CACHE["nc"] = build_nc()
    nc = _CACHE["nc"]

    x = np.ascontiguousarray(np.asarray(x, dtype=np.float32)).reshape(B, C, N)
    common = {
        "gamma": np.asarray(gamma, np.float32),
        "beta": np.asarray(beta, np.float32),
        "Wq": np.asarray(Wq, np.float32),
        "bq": np.asarray(bq, np.float32),
        "Wk": np.asarray(Wk, np.float32),
        "bk": np.asarray(bk, np.float32),
        "Wv": np.asarray(Wv, np.float32),
        "bv": np.asarray(bv, np.float32),
        "Wp": np.asarray(Wp, np.float32),
        "bp": np.asarray(bp, np.float32),
    }
    in_maps = [{"x": x[b], **common} for b in range(B)]
    res = bass_utils.run_bass_kernel_spmd(nc, in_maps, core_ids=list(range(B)))
    out = np.stack([res.results[b]["out"] for b in range(B)])
    return out.reshape(B, C, H, W)    if not T8_INLINE:
        for cb in range(CB):
            s_col, b_col = gn_cols[cb]
            for k in range(N // 1024):
                asl = slice(k * 1024, (k + 1) * 1024)
                eng = T8_SCHED[cb][k]
                if eng == "D":
                    nc.vector.tensor_scalar(
                        out=t8[:, cb, asl], in0=t_cm[:, cb, asl], scalar1=s_col,
                        scalar2=b_col, op0=OP.mult, op1=OP.add,
                    )
                elif eng == "P":
                    nc.gpsimd.tensor_scalar(
                        out=t8[:, cb, asl], in0=t_cm[:, cb, asl], scalar1=s_col,
                        scalar2=b_col, op0=OP.mult, op1=OP.add,
                    )
                else:
                    nc.scalar.activation(
                        out=t8[:, cb, asl], in_=t_cm[:, cb, asl], func=AF.Identity,
                        bias=b_col, scale=s_col,
                    )


